# revision 46
# baseline (speedup 1.0000x reference)
"""Batched ragged segment-mean (BERTEmbedder merge loop) on 8 TRN2 NeuronCores.

Strategy
--------
Data-parallel over the batch: each of the 8 cores processes 2 of the 16
sequences (assignment chosen by the host, see below).  Within a sequence,
segment-sum is computed as a block-sparse one-hot matmul on the PE:

    out[t, d] = sum_s onehot[s, t] * x[s, d]

Segment ids are sorted per row, so each 128-subtoken tile only covers a
narrow window of token ids.  The host inspects the ids and builds a static
(s_tile, t_tile) pair schedule: matmuls are emitted only into the 128-row
t-tiles each s-tile's ids can touch (union over the sequences that share
the SPMD program slot, so one program serves all 8 cores).  A column of
ones (appended to the input rows by the host) accumulates per-token counts
in the same PSUM tile; a reciprocal-multiply turns sums into means.

The harness gate is rel_err < 2e-2, which buys two precision tricks
(measured 4.3e-3 end to end):
 * the input is host-split into hi = fp8_e4m3(x) and lo = fp8_e4m3(x - hi)
   (reconstruction error ~0.4%).  A single fp8 DoubleRow matmul contracts
   BOTH planes at once (256-row contraction): the Ko=2 interleave dim
   walks the hi/lo planes of the x tile, while the one-hot weights use a
   stride-0 broadcast so both planes see the same 128x128 one-hot.  One
   matmul pair (and one weight load) per (s_tile, t_tile) does what the
   fp32 hi/lo version needed four of, and the lighter PE duty also keeps
   the PE out of the P0 power downclock (2.4 vs 2.0 GHz measured).
 * the output lands in HBM as bf16 (halves the write) before the host
   widens it back to fp32.
Per-core HBM traffic is ~19.3 MB vs 37.8 MB for fp32.  PSUM accumulates
fp32, so the ones-column counts stay exact.  The one-hots are built on the
DVE in a packed uint16 form (0x0038 / 0x3800 = fp8 pairs (1,0) / (0,1)) --
one fast 2-byte-dtype compare-multiply per s-tile against a static iota,
with floor(sid/2) and the sid-parity selector precomputed on the host and
DMA'd in as one small tensor per slot (the direct fp8 DVE write path is
~13x slower; uint16 then bitcast avoids it).  Input loads issue from the
Sync HWDGE queue and output stores from the Scalar HWDGE queue so
output-drain waits never head-of-line-block input prefetch; the first
input group is split in half so the first matmuls start ~1us earlier, and
outputs drain in 2-t-tile chunks to keep store bursts from starving the
loads (input+output peak demand exceeds the ~358 GB/s per-core HBM limit).
A short dummy-matmul chain at program start trips the PE HAM activity
window so real matmuls start at full clock.  The 16 sequences are assigned
to the two SPMD program slots by searching all 6435 8/8 partitions for the
one minimizing total union-schedule pairs.
"""

import os
import numpy as np

B, S, D, T, P = 16, 4096, 768, 2048, 128
NCORES = 8
SPC = B // NCORES          # sequences per core
NST, NTT = S // P, T // P  # 32 s-tiles, 16 t-tiles
DSPLIT = 512               # PSUM bank limit (fp32 words)
DW = 784                   # 768 data + ones col + pad to a 16-elem multiple
SUPER = 4                  # s-tiles per x-load DMA group
OG = 2                     # t-tiles per output-store DMA

_cache: dict = {}


def _schedule(segment_ids: np.ndarray):
    """Per program slot q: which t-tiles each s-tile touches, unioned over
    the sequences that run in that slot on every core (SPMD)."""
    from itertools import combinations
    mins = segment_ids.reshape(B, NST, P).min(2) // P
    maxs = segment_ids.reshape(B, NST, P).max(2) // P

    def _npairs(group):
        return int((maxs[list(group)].max(0) - mins[list(group)].min(0) + 1).sum())

    best = None
    allseq = set(range(B))
    for combo in combinations(range(1, B), NCORES - 1):
        g0 = (0,) + combo
        g1 = tuple(sorted(allseq - set(g0)))
        c = _npairs(g0) + _npairs(g1)
        if best is None or c < best[0]:
            best = (c, (g0, g1))
    slot_seqs = best[1]

    sched = []
    for q in range(SPC):
        seqs = list(slot_seqs[q])
        js_of = []
        for i in range(NST):
            blk = segment_ids[seqs, i * P:(i + 1) * P]
            lo, hi = int(blk.min()), int(blk.max())
            js_of.append(list(range(lo // P, hi // P + 1)))
        first, last = {}, {}
        for i in range(NST):
            for j in js_of[i]:
                first.setdefault(j, i)
                last[j] = i
        # loud guard: the PSUM accumulator pools have 4 slots each; more
        # simultaneously-open t-tiles would deadlock the tile scheduler
        maxopen = max(sum(1 for j in first if first[j] <= i <= last[j])
                      for i in range(NST))
        assert maxopen <= 3, f"schedule needs {maxopen} open PSUM accumulators"
        sched.append((tuple(tuple(js) for js in js_of),
                      tuple(sorted(first.items())),
                      tuple(sorted(last.items()))))
    return tuple(sched), slot_seqs


def _build(sched):
    from contextlib import ExitStack
    import concourse.bacc as bacc
    import concourse.tile as tile
    import concourse.mybir as mybir

    f32, f16 = mybir.dt.float32, mybir.dt.float16
    bf16, i32, f8 = mybir.dt.bfloat16, mybir.dt.int32, mybir.dt.float8e4
    u16 = mybir.dt.uint16
    AO = mybir.AluOpType
    DR = mybir.MatmulPerfMode.DoubleRow
    nc = bacc.Bacc("TRN2", target_bir_lowering=False, debug=False)
    # hi = fp8(x), lo = fp8(x - hi); col 768 is 1.0 in the hi plane only
    # (counts accumulate once), cols 769.. are zero pad
    xh_d = nc.dram_tensor("x_hi", [SPC, S, DW], f8, kind="ExternalInput").ap()
    xl_d = nc.dram_tensor("x_lo", [SPC, S, DW], f8, kind="ExternalInput").ap()
    # host-precomputed one-hot ingredients, per s-tile column i:
    # sv[.., 0:NST]  srel = floor(sid/2) - 64 * first t-tile of i's window
    # sv[.., NST:]   vsel = 56 if sid even else 14336 (uint16 bit patterns
    #                of the packed fp8e4 pairs (1.0, 0) / (0, 1.0))
    sv_d = nc.dram_tensor("sv", [SPC, P, 2 * NST], f32,
                          kind="ExternalInput").ap()
    out = nc.dram_tensor("out", [SPC, T, D], bf16, kind="ExternalOutput").ap()

    with ExitStack() as ctx:
        tc = ctx.enter_context(tile.TileContext(nc))
        const = ctx.enter_context(tc.tile_pool(name="const", bufs=1))
        xp = ctx.enter_context(tc.tile_pool(name="xp", bufs=8))
        ohp = ctx.enter_context(tc.tile_pool(name="ohp", bufs=12))
        outp = ctx.enter_context(tc.tile_pool(name="outp", bufs=3))
        smp = ctx.enter_context(tc.tile_pool(name="smp", bufs=4))
        psb = ctx.enter_context(tc.tile_pool(name="psb", bufs=4, space="PSUM"))

        maxw = P * max(len(js) for q in range(SPC) for js in sched[q][0])
        ws = const.tile([P, DSPLIT], bf16)
        nc.gpsimd.memset(ws[:], 0.0)
        iota_i = const.tile([P, maxw // 2], i32)
        nc.gpsimd.iota(iota_i[:], pattern=[[1, maxw // 2]], base=0,
                       channel_multiplier=0)
        iota2_h = const.tile([P, maxw // 2], f16)
        nc.vector.tensor_copy(iota2_h[:], iota_i[:])

        # dummy accumulation chain: ~2us of PE activity while the first x
        # tiles are still in flight trips the HAM activity monitor, so the
        # real matmuls start at 2.4 GHz instead of the 1.2 GHz cold clock
        wps = psb.tile([P, DSPLIT], f32, tag="psA", name="warm")
        for k in range(5):
            nc.tensor.matmul(wps[:], lhsT=ws[:, 0:P], rhs=ws[:],
                             start=(k == 0), stop=(k == 4))

        # one-hot ingredients come precomputed from the host: one small DMA
        # per slot instead of a sid-transpose + 7-op DVE chain on the
        # critical path to the first matmul
        srels, vsels = [], []
        for q in range(SPC):
            sv_t = smp.tile([P, 2 * NST], f32, tag="sv", name=f"sv_{q}")
            nc.sync.dma_start(out=sv_t[:], in_=sv_d[q])
            srels.append(sv_t[:, 0:NST])
            vsels.append(sv_t[:, NST:2 * NST])

        ctxs = []
        for q in range(SPC):
            js_of, first_t, last_t = sched[q]
            ctxs.append({
                "js_of": js_of, "first": dict(first_t), "last": dict(last_t),
                "srel": srels[q], "vsel": vsels[q],
                "xh_seq": xh_d[q].rearrange("(n p) d -> p n d", p=P),
                "xl_seq": xl_d[q].rearrange("(n p) d -> p n d", p=P),
                "out_seq": out[q].rearrange("(n p) d -> p n d", p=P),
                "open_ps": {}, "pend_out": {}})

        def emit_group(q, g):
            c = ctxs[q]
            js_of, first, last = c["js_of"], c["first"], c["last"]
            srel, vsel = c["srel"], c["vsel"]
            open_ps, pend_out = c["open_ps"], c["pend_out"]
            xt = xp.tile([P, 2, SUPER, DW], f8, tag="xt", name=f"xt_q{q}_g{g}")
            if g == 0:
                # split the first group's loads so the first matmuls only
                # wait on half the data -- shortens the pipeline head
                h = SUPER // 2
                nc.sync.dma_start(out=xt[:, 0, 0:h],
                                  in_=c["xh_seq"][:, 0:h, :])
                nc.sync.dma_start(out=xt[:, 1, 0:h],
                                  in_=c["xl_seq"][:, 0:h, :])
                nc.sync.dma_start(out=xt[:, 0, h:SUPER],
                                  in_=c["xh_seq"][:, h:SUPER, :])
                nc.sync.dma_start(out=xt[:, 1, h:SUPER],
                                  in_=c["xl_seq"][:, h:SUPER, :])
            else:
                nsl = slice(g * SUPER, (g + 1) * SUPER)
                nc.sync.dma_start(out=xt[:, 0], in_=c["xh_seq"][:, nsl, :])
                nc.sync.dma_start(out=xt[:, 1], in_=c["xl_seq"][:, nsl, :])
            # one-hots first: they only depend on the sid precompute, so
            # the DVE can produce them while the x DMA is still in flight
            ohws = []
            for l in range(SUPER):
                i = g * SUPER + l
                hw = P * len(js_of[i]) // 2
                ohw = ohp.tile([P, hw], u16, tag="oh", name=f"oh_q{q}_i{i}")
                nc.vector.tensor_scalar(
                    ohw[:], iota2_h[:, 0:hw], srel[:, i:i + 1],
                    vsel[:, i:i + 1], AO.is_equal, AO.mult)
                ohws.append(ohw)
            # one fp8 DoubleRow matmul pair per (s_tile, t_tile): Ko walks
            # the hi/lo planes of x; the one-hot broadcasts across Ko
            for l in range(SUPER):
                i = g * SUPER + l
                for k, j in enumerate(js_of[i]):
                    st = first[j] == i
                    sp_ = last[j] == i
                    if st:
                        open_ps[j] = (
                            psb.tile([P, DSPLIT], f32, tag="psA",
                                     name=f"accA_q{q}_j{j}"),
                            psb.tile([P, DW - DSPLIT], f32, tag="psB",
                                     name=f"accB_q{q}_j{j}"))
                    pa, pb = open_ps[j]
                    oh = (ohws[l][:, k * P // 2:(k + 1) * P // 2]
                          .bitcast(f8)
                          .rearrange("p (o w) -> p o w", o=1)
                          .broadcast_to((P, 2, P)))
                    nc.tensor.matmul(pa[:], lhsT=oh,
                                     rhs=xt[:, :, l, 0:DSPLIT],
                                     start=st, stop=sp_, perf_mode=DR)
                    nc.tensor.matmul(pb[:], lhsT=oh,
                                     rhs=xt[:, :, l, DSPLIT:DW],
                                     start=st, stop=sp_, perf_mode=DR)
                    if sp_:
                        cnt = smp.tile([P, 1], f32, tag="cnt")
                        nc.vector.tensor_scalar_max(
                            cnt[:], pb[:, D - DSPLIT:D - DSPLIT + 1], 1.0)
                        rec = smp.tile([P, 1], f32, tag="rec")
                        nc.vector.reciprocal(rec[:], cnt[:])
                        jp = j // OG
                        if jp not in pend_out:
                            ot = outp.tile([P, OG, D], bf16, tag="ot",
                                           name=f"ot_q{q}_{jp}")
                            need = 0
                            for m in range(OG):
                                if jp * OG + m in first:
                                    need += 1
                                else:
                                    nc.vector.memset(ot[:, m, :], 0.0)
                            pend_out[jp] = [ot, need]
                        ot, _ = pend_out[jp]
                        sl = j % OG
                        nc.scalar.activation(ot[:, sl, 0:DSPLIT], pa[:],
                                             mybir.ActivationFunctionType.Copy,
                                             scale=rec[:])
                        nc.vector.tensor_scalar_mul(
                            ot[:, sl, DSPLIT:D], pb[:, 0:D - DSPLIT], rec[:])
                        pend_out[jp][1] -= 1
                        if pend_out[jp][1] == 0:
                            nc.scalar.dma_start(
                                out=c["out_seq"][:, OG * jp:OG * (jp + 1), :],
                                in_=ot[:])
                            del pend_out[jp]
                        del open_ps[j]

        # interleave the two slots' groups: two independent dependency
        # chains keep every engine fed through the other chain's stalls
        for g in range(NST // SUPER):
            for q in range(SPC):
                emit_group(q, g)

        for q in range(SPC):
            c = ctxs[q]
            first, out_seq = c["first"], c["out_seq"]
            assert not c["pend_out"], "output group left pending"
            # output groups no s-tile can touch: store zeros
            for jp in range(NTT // OG):
                if all(jp * OG + m not in first for m in range(OG)):
                    zt = outp.tile([P, OG, D], bf16, tag="ot",
                                   name=f"zt_q{q}_{jp}")
                    nc.vector.memset(zt[:], 0.0)
                    nc.scalar.dma_start(
                        out=out_seq[:, OG * jp:OG * (jp + 1), :], in_=zt[:])
    nc.compile()
    return nc


def _get_nc(segment_ids: np.ndarray):
    sched, slot_seqs = _schedule(segment_ids)
    if sched not in _cache:
        _cache[sched] = _build(sched)
    return _cache[sched], slot_seqs, sched


def run(raw_output, segment_ids, trace=False):
    import ml_dtypes
    from concourse.bass_utils import run_bass_kernel_spmd

    f8 = ml_dtypes.float8_e4m3
    raw_output = np.asarray(raw_output, dtype=np.float32)
    segment_ids = np.ascontiguousarray(segment_ids, dtype=np.int32)
    nc, slot_seqs, sched = _get_nc(segment_ids)
    hi = raw_output.astype(f8)
    lo = (raw_output - hi.astype(np.float32)).astype(f8)
    x_hi = np.zeros((B, S, DW), dtype=f8)
    x_hi[:, :, 0:D] = hi
    x_hi[:, :, D] = 1.0
    x_lo = np.zeros((B, S, DW), dtype=f8)
    x_lo[:, :, 0:D] = lo
    wb = np.empty((SPC, NST), dtype=np.int64)
    for q in range(SPC):
        js_of = sched[q][0]
        for i in range(NST):
            wb[q, i] = js_of[i][0] * 64
    in_maps = []
    for c in range(NCORES):
        seqs = [slot_seqs[q][c] for q in range(SPC)]
        sv = np.empty((SPC, P, 2 * NST), dtype=np.float32)
        for q in range(SPC):
            sidr = segment_ids[seqs[q]].reshape(NST, P)        # [i, p]
            sv[q, :, 0:NST] = (sidr // 2 - wb[q][:, None]).T
            sv[q, :, NST:] = np.where(sidr % 2 == 0, 56.0, 14336.0).T
        in_maps.append({
            "x_hi": np.ascontiguousarray(x_hi[seqs]),
            "x_lo": np.ascontiguousarray(x_lo[seqs]),
            "sv": sv})
    bkr = run_bass_kernel_spmd(nc, in_maps, list(range(NCORES)), trace=trace)
    full = np.empty((B, T, D), np.float32)
    for c in range(NCORES):
        for q in range(SPC):
            full[slot_seqs[q][c]] = bkr.results[c]["out"][q].astype(np.float32)
    return full, bkr


def kernel(raw_output, segment_ids):
    full, _ = run(raw_output, segment_ids,
                  trace=bool(int(os.environ.get("KERNEL_TRACE", "0"))))
    return full


# revision 47
# speedup vs baseline: 1.0133x; 1.0133x over previous
"""Batched ragged segment-mean (BERTEmbedder merge loop) on 8 TRN2 NeuronCores.

Strategy
--------
Data-parallel over the batch: each of the 8 cores processes 2 of the 16
sequences (assignment chosen by the host, see below).  Within a sequence,
segment-sum is computed as a block-sparse one-hot matmul on the PE:

    out[t, d] = sum_s onehot[s, t] * x[s, d]

Segment ids are sorted per row, so each 128-subtoken tile only covers a
narrow window of token ids.  The host inspects the ids and builds a static
(s_tile, t_tile) pair schedule: matmuls are emitted only into the 128-row
t-tiles each s-tile's ids can touch (union over the sequences that share
the SPMD program slot, so one program serves all 8 cores).  A column of
ones (appended to the input rows by the host) accumulates per-token counts
in the same PSUM tile; a reciprocal-multiply turns sums into means.

The harness gate is rel_err < 2e-2, which buys two precision tricks
(measured 4.3e-3 end to end):
 * the input is host-split into hi = fp8_e4m3(x) and lo = fp8_e4m3(x - hi)
   (reconstruction error ~0.4%).  A single fp8 DoubleRow matmul contracts
   BOTH planes at once (256-row contraction): the Ko=2 interleave dim
   walks the hi/lo planes of the x tile, while the one-hot weights use a
   stride-0 broadcast so both planes see the same 128x128 one-hot.  One
   matmul pair (and one weight load) per (s_tile, t_tile) does what the
   fp32 hi/lo version needed four of, and the lighter PE duty also keeps
   the PE out of the P0 power downclock (2.4 vs 2.0 GHz measured).
 * the output lands in HBM as bf16 (halves the write) before the host
   widens it back to fp32.
Per-core HBM traffic is ~19.3 MB vs 37.8 MB for fp32.  PSUM accumulates
fp32, so the ones-column counts stay exact.  The one-hots are built on the
DVE in a packed uint16 form (0x0038 / 0x3800 = fp8 pairs (1,0) / (0,1)) --
one fast 2-byte-dtype compare-multiply per s-tile against a static iota,
with floor(sid/2) and the sid-parity selector precomputed on the host and
DMA'd in as one small tensor per slot (the direct fp8 DVE write path is
~13x slower; uint16 then bitcast avoids it).  Input loads issue from the
Sync HWDGE queue and output stores from the Scalar HWDGE queue so
output-drain waits never head-of-line-block input prefetch; the first
input group is split in half so the first matmuls start ~1us earlier, and
outputs drain in 2-t-tile chunks to keep store bursts from starving the
loads (input+output peak demand exceeds the ~358 GB/s per-core HBM limit).
A short dummy-matmul chain at program start trips the PE HAM activity
window so real matmuls start at full clock.  The 16 sequences are assigned
to the two SPMD program slots by searching all 6435 8/8 partitions for the
one minimizing total union-schedule pairs.
"""

import os
import numpy as np

B, S, D, T, P = 16, 4096, 768, 2048, 128
NCORES = 8
SPC = B // NCORES          # sequences per core
NST, NTT = S // P, T // P  # 32 s-tiles, 16 t-tiles
DSPLIT = 512               # PSUM bank limit (fp32 words)
DW = 784                   # 768 data + ones col + pad to a 16-elem multiple
SUPER = 4                  # s-tiles per x-load DMA group
OG = 2                     # t-tiles per output-store DMA

_cache: dict = {}


def _schedule(segment_ids: np.ndarray):
    """Per program slot q: which t-tiles each s-tile touches, unioned over
    the sequences that run in that slot on every core (SPMD)."""
    from itertools import combinations
    mins = segment_ids.reshape(B, NST, P).min(2) // P
    maxs = segment_ids.reshape(B, NST, P).max(2) // P

    def _npairs(group):
        return int((maxs[list(group)].max(0) - mins[list(group)].min(0) + 1).sum())

    cands = []
    allseq = set(range(B))
    for combo in combinations(range(1, B), NCORES - 1):
        g0 = (0,) + combo
        g1 = tuple(sorted(allseq - set(g0)))
        cands.append((_npairs(g0) + _npairs(g1), (g0, g1)))
    cands.sort()

    def _try(slot_seqs):
        sched = []
        for q in range(SPC):
            seqs = list(slot_seqs[q])
            js_of = []
            for i in range(NST):
                blk = segment_ids[seqs, i * P:(i + 1) * P]
                lo, hi = int(blk.min()), int(blk.max())
                js_of.append(list(range(lo // P, hi // P + 1)))
            first, last = {}, {}
            for i in range(NST):
                for j in js_of[i]:
                    first.setdefault(j, i)
                    last[j] = i
            # the PSUM accumulator pools have 4 slots each; more
            # simultaneously-open t-tiles would deadlock the tile scheduler
            maxopen = max(sum(1 for j in first if first[j] <= i <= last[j])
                          for i in range(NST))
            if maxopen > 3:
                return None
            sched.append((tuple(tuple(js) for js in js_of),
                          tuple(sorted(first.items())),
                          tuple(sorted(last.items()))))
        return tuple(sched)

    for _, slot_seqs in cands:
        sched = _try(slot_seqs)
        if sched is not None:
            return sched, slot_seqs
    raise RuntimeError("no slot partition fits 3 open PSUM accumulators")


def _build(sched):
    from contextlib import ExitStack
    import concourse.bacc as bacc
    import concourse.tile as tile
    import concourse.mybir as mybir

    f32, f16 = mybir.dt.float32, mybir.dt.float16
    bf16, i32, f8 = mybir.dt.bfloat16, mybir.dt.int32, mybir.dt.float8e4
    u16 = mybir.dt.uint16
    AO = mybir.AluOpType
    DR = mybir.MatmulPerfMode.DoubleRow
    nc = bacc.Bacc("TRN2", target_bir_lowering=False, debug=False)
    # hi = fp8(x), lo = fp8(x - hi); col 768 is 1.0 in the hi plane only
    # (counts accumulate once), cols 769.. are zero pad
    xh_d = nc.dram_tensor("x_hi", [SPC, S, DW], f8, kind="ExternalInput").ap()
    xl_d = nc.dram_tensor("x_lo", [SPC, S, DW], f8, kind="ExternalInput").ap()
    # host-precomputed one-hot ingredients, per s-tile column i:
    # sv[.., 0:NST]  srel = floor(sid/2) - 64 * first t-tile of i's window
    # sv[.., NST:]   vsel = 56 if sid even else 14336 (uint16 bit patterns
    #                of the packed fp8e4 pairs (1.0, 0) / (0, 1.0))
    sv_d = nc.dram_tensor("sv", [SPC, P, 2 * NST], f32,
                          kind="ExternalInput").ap()
    out = nc.dram_tensor("out", [SPC, T, D], bf16, kind="ExternalOutput").ap()

    with ExitStack() as ctx:
        tc = ctx.enter_context(tile.TileContext(nc))
        const = ctx.enter_context(tc.tile_pool(name="const", bufs=1))
        xp = ctx.enter_context(tc.tile_pool(name="xp", bufs=8))
        ohp = ctx.enter_context(tc.tile_pool(name="ohp", bufs=12))
        outp = ctx.enter_context(tc.tile_pool(name="outp", bufs=3))
        smp = ctx.enter_context(tc.tile_pool(name="smp", bufs=4))
        psb = ctx.enter_context(tc.tile_pool(name="psb", bufs=4, space="PSUM"))

        maxw = P * max(len(js) for q in range(SPC) for js in sched[q][0])
        ws = const.tile([P, DSPLIT], bf16)
        nc.gpsimd.memset(ws[:], 0.0)
        iota_i = const.tile([P, maxw // 2], i32)
        nc.gpsimd.iota(iota_i[:], pattern=[[1, maxw // 2]], base=0,
                       channel_multiplier=0)
        iota2_h = const.tile([P, maxw // 2], f16)
        nc.vector.tensor_copy(iota2_h[:], iota_i[:])

        # dummy accumulation chain: ~2us of PE activity while the first x
        # tiles are still in flight trips the HAM activity monitor, so the
        # real matmuls start at 2.4 GHz instead of the 1.2 GHz cold clock
        wps = psb.tile([P, DSPLIT], f32, tag="psA", name="warm")
        for k in range(5):
            nc.tensor.matmul(wps[:], lhsT=ws[:, 0:P], rhs=ws[:],
                             start=(k == 0), stop=(k == 4))

        # one-hot ingredients come precomputed from the host: one small DMA
        # per slot instead of a sid-transpose + 7-op DVE chain on the
        # critical path to the first matmul
        srels, vsels = [], []
        for q in range(SPC):
            sv_t = smp.tile([P, 2 * NST], f32, tag="sv", name=f"sv_{q}")
            nc.sync.dma_start(out=sv_t[:], in_=sv_d[q])
            srels.append(sv_t[:, 0:NST])
            vsels.append(sv_t[:, NST:2 * NST])

        ctxs = []
        for q in range(SPC):
            js_of, first_t, last_t = sched[q]
            ctxs.append({
                "js_of": js_of, "first": dict(first_t), "last": dict(last_t),
                "srel": srels[q], "vsel": vsels[q],
                "xh_seq": xh_d[q].rearrange("(n p) d -> p n d", p=P),
                "xl_seq": xl_d[q].rearrange("(n p) d -> p n d", p=P),
                "out_seq": out[q].rearrange("(n p) d -> p n d", p=P),
                "open_ps": {}, "pend_out": {}})

        def emit_group(q, g):
            c = ctxs[q]
            js_of, first, last = c["js_of"], c["first"], c["last"]
            srel, vsel = c["srel"], c["vsel"]
            open_ps, pend_out = c["open_ps"], c["pend_out"]
            xt = xp.tile([P, 2, SUPER, DW], f8, tag="xt", name=f"xt_q{q}_g{g}")
            if g == 0:
                # split the first group's loads so the first matmuls only
                # wait on half the data -- shortens the pipeline head
                h = SUPER // 2
                nc.sync.dma_start(out=xt[:, 0, 0:h],
                                  in_=c["xh_seq"][:, 0:h, :])
                nc.sync.dma_start(out=xt[:, 1, 0:h],
                                  in_=c["xl_seq"][:, 0:h, :])
                nc.sync.dma_start(out=xt[:, 0, h:SUPER],
                                  in_=c["xh_seq"][:, h:SUPER, :])
                nc.sync.dma_start(out=xt[:, 1, h:SUPER],
                                  in_=c["xl_seq"][:, h:SUPER, :])
            else:
                nsl = slice(g * SUPER, (g + 1) * SUPER)
                nc.sync.dma_start(out=xt[:, 0], in_=c["xh_seq"][:, nsl, :])
                nc.sync.dma_start(out=xt[:, 1], in_=c["xl_seq"][:, nsl, :])
            # one-hots first: they only depend on the sid precompute, so
            # the DVE can produce them while the x DMA is still in flight
            ohws = []
            for l in range(SUPER):
                i = g * SUPER + l
                hw = P * len(js_of[i]) // 2
                ohw = ohp.tile([P, hw], u16, tag="oh", name=f"oh_q{q}_i{i}")
                nc.vector.tensor_scalar(
                    ohw[:], iota2_h[:, 0:hw], srel[:, i:i + 1],
                    vsel[:, i:i + 1], AO.is_equal, AO.mult)
                ohws.append(ohw)
            # one fp8 DoubleRow matmul pair per (s_tile, t_tile): Ko walks
            # the hi/lo planes of x; the one-hot broadcasts across Ko
            for l in range(SUPER):
                i = g * SUPER + l
                for k, j in enumerate(js_of[i]):
                    st = first[j] == i
                    sp_ = last[j] == i
                    if st:
                        open_ps[j] = (
                            psb.tile([P, DSPLIT], f32, tag="psA",
                                     name=f"accA_q{q}_j{j}"),
                            psb.tile([P, DW - DSPLIT], f32, tag="psB",
                                     name=f"accB_q{q}_j{j}"))
                    pa, pb = open_ps[j]
                    oh = (ohws[l][:, k * P // 2:(k + 1) * P // 2]
                          .bitcast(f8)
                          .rearrange("p (o w) -> p o w", o=1)
                          .broadcast_to((P, 2, P)))
                    nc.tensor.matmul(pa[:], lhsT=oh,
                                     rhs=xt[:, :, l, 0:DSPLIT],
                                     start=st, stop=sp_, perf_mode=DR)
                    nc.tensor.matmul(pb[:], lhsT=oh,
                                     rhs=xt[:, :, l, DSPLIT:DW],
                                     start=st, stop=sp_, perf_mode=DR)
                    if sp_:
                        cnt = smp.tile([P, 1], f32, tag="cnt")
                        nc.vector.tensor_scalar_max(
                            cnt[:], pb[:, D - DSPLIT:D - DSPLIT + 1], 1.0)
                        rec = smp.tile([P, 1], f32, tag="rec")
                        nc.vector.reciprocal(rec[:], cnt[:])
                        jp = j // OG
                        if jp not in pend_out:
                            ot = outp.tile([P, OG, D], bf16, tag="ot",
                                           name=f"ot_q{q}_{jp}")
                            need = 0
                            for m in range(OG):
                                if jp * OG + m in first:
                                    need += 1
                                else:
                                    nc.vector.memset(ot[:, m, :], 0.0)
                            pend_out[jp] = [ot, need]
                        ot, _ = pend_out[jp]
                        sl = j % OG
                        nc.scalar.activation(ot[:, sl, 0:DSPLIT], pa[:],
                                             mybir.ActivationFunctionType.Copy,
                                             scale=rec[:])
                        nc.vector.tensor_scalar_mul(
                            ot[:, sl, DSPLIT:D], pb[:, 0:D - DSPLIT], rec[:])
                        pend_out[jp][1] -= 1
                        if pend_out[jp][1] == 0:
                            nc.scalar.dma_start(
                                out=c["out_seq"][:, OG * jp:OG * (jp + 1), :],
                                in_=ot[:])
                            del pend_out[jp]
                        del open_ps[j]

        # interleave the two slots' groups: two independent dependency
        # chains keep every engine fed through the other chain's stalls
        for g in range(NST // SUPER):
            for q in range(SPC):
                emit_group(q, g)

        for q in range(SPC):
            c = ctxs[q]
            first, out_seq = c["first"], c["out_seq"]
            assert not c["pend_out"], "output group left pending"
            # output groups no s-tile can touch: store zeros
            for jp in range(NTT // OG):
                if all(jp * OG + m not in first for m in range(OG)):
                    zt = outp.tile([P, OG, D], bf16, tag="ot",
                                   name=f"zt_q{q}_{jp}")
                    nc.vector.memset(zt[:], 0.0)
                    nc.scalar.dma_start(
                        out=out_seq[:, OG * jp:OG * (jp + 1), :], in_=zt[:])
    nc.compile()
    return nc


def _get_nc(segment_ids: np.ndarray):
    sched, slot_seqs = _schedule(segment_ids)
    if sched not in _cache:
        _cache[sched] = _build(sched)
    return _cache[sched], slot_seqs, sched


def run(raw_output, segment_ids, trace=False):
    import ml_dtypes
    from concourse.bass_utils import run_bass_kernel_spmd

    f8 = ml_dtypes.float8_e4m3
    raw_output = np.asarray(raw_output, dtype=np.float32)
    segment_ids = np.ascontiguousarray(segment_ids, dtype=np.int32)
    nc, slot_seqs, sched = _get_nc(segment_ids)
    hi = raw_output.astype(f8)
    lo = (raw_output - hi.astype(np.float32)).astype(f8)
    x_hi = np.zeros((B, S, DW), dtype=f8)
    x_hi[:, :, 0:D] = hi
    x_hi[:, :, D] = 1.0
    x_lo = np.zeros((B, S, DW), dtype=f8)
    x_lo[:, :, 0:D] = lo
    wb = np.empty((SPC, NST), dtype=np.int64)
    for q in range(SPC):
        js_of = sched[q][0]
        for i in range(NST):
            wb[q, i] = js_of[i][0] * 64
    in_maps = []
    for c in range(NCORES):
        seqs = [slot_seqs[q][c] for q in range(SPC)]
        sv = np.empty((SPC, P, 2 * NST), dtype=np.float32)
        for q in range(SPC):
            sidr = segment_ids[seqs[q]].reshape(NST, P)        # [i, p]
            sv[q, :, 0:NST] = (sidr // 2 - wb[q][:, None]).T
            sv[q, :, NST:] = np.where(sidr % 2 == 0, 56.0, 14336.0).T
        in_maps.append({
            "x_hi": np.ascontiguousarray(x_hi[seqs]),
            "x_lo": np.ascontiguousarray(x_lo[seqs]),
            "sv": sv})
    bkr = run_bass_kernel_spmd(nc, in_maps, list(range(NCORES)), trace=trace)
    full = np.empty((B, T, D), np.float32)
    for c in range(NCORES):
        for q in range(SPC):
            full[slot_seqs[q][c]] = bkr.results[c]["out"][q].astype(np.float32)
    return full, bkr


def kernel(raw_output, segment_ids):
    full, _ = run(raw_output, segment_ids,
                  trace=bool(int(os.environ.get("KERNEL_TRACE", "0"))))
    return full


# revision 54
# speedup vs baseline: 1.0204x; 1.0070x over previous
"""Batched ragged segment-mean (BERTEmbedder merge loop) on 8 TRN2 NeuronCores.

Strategy
--------
Data-parallel over the batch: each of the 8 cores processes 2 of the 16
sequences (assignment chosen by the host, see below).  Within a sequence,
segment-sum is computed as a block-sparse one-hot matmul on the PE:

    out[t, d] = sum_s onehot[s, t] * x[s, d]

Segment ids are sorted per row, so each 128-subtoken tile only covers a
narrow window of token ids.  The host inspects the ids and builds a static
(s_tile, t_tile) pair schedule: matmuls are emitted only into the 128-row
t-tiles each s-tile's ids can touch (union over the sequences that share
the SPMD program slot, so one program serves all 8 cores).  A column of
ones (appended to the input rows by the host) accumulates per-token counts
in the same PSUM tile; a reciprocal-multiply turns sums into means.

The harness gate is rel_err < 2e-2, which buys two precision tricks
(measured 4.3e-3 end to end):
 * the input is host-split into hi = fp8_e4m3(x) and lo = fp8_e4m3(x - hi)
   (reconstruction error ~0.4%).  A single fp8 DoubleRow matmul contracts
   BOTH planes at once (256-row contraction): the Ko=2 interleave dim
   walks the hi/lo planes of the x tile, while the one-hot weights use a
   stride-0 broadcast so both planes see the same 128x128 one-hot.  One
   matmul pair (and one weight load) per (s_tile, t_tile) does what the
   fp32 hi/lo version needed four of, and the lighter PE duty also keeps
   the PE out of the P0 power downclock (2.4 vs 2.0 GHz measured).
 * the output lands in HBM as bf16 (halves the write) before the host
   widens it back to fp32.
Per-core HBM traffic is ~19.3 MB vs 37.8 MB for fp32.  PSUM accumulates
fp32, so the ones-column counts stay exact.  The one-hots are built on the
DVE in a packed uint16 form (0x0038 / 0x3800 = fp8 pairs (1,0) / (0,1)) --
one fast 2-byte-dtype compare-multiply per s-tile against a static iota,
with floor(sid/2) and the sid-parity selector precomputed on the host and
DMA'd in as one small tensor per slot (the direct fp8 DVE write path is
~13x slower; uint16 then bitcast avoids it).  Input loads issue from the
Sync HWDGE queue and output stores from the Scalar HWDGE queue so
output-drain waits never head-of-line-block input prefetch; the first
input group is split in half so the first matmuls start ~1us earlier, and
outputs drain in 2-t-tile chunks to keep store bursts from starving the
loads (input+output peak demand exceeds the ~358 GB/s per-core HBM limit).
A short dummy-matmul chain at program start trips the PE HAM activity
window so real matmuls start at full clock.  The 16 sequences are assigned
to the two SPMD program slots by searching all 6435 8/8 partitions for the
one minimizing total union-schedule pairs.
"""

import os
import numpy as np

B, S, D, T, P = 16, 4096, 768, 2048, 128
NCORES = 8
SPC = B // NCORES          # sequences per core
NST, NTT = S // P, T // P  # 32 s-tiles, 16 t-tiles
DSPLIT = 512               # PSUM bank limit (fp32 words)
DW = 768                   # data columns only (counts are host-precomputed)
SUPER = 4                  # s-tiles per x-load DMA group
OG = 2                     # t-tiles per output-store DMA

_cache: dict = {}


def _schedule(segment_ids: np.ndarray):
    """Per program slot q: which t-tiles each s-tile touches, unioned over
    the sequences that run in that slot on every core (SPMD)."""
    from itertools import combinations
    mins = segment_ids.reshape(B, NST, P).min(2) // P
    maxs = segment_ids.reshape(B, NST, P).max(2) // P

    def _npairs(group):
        return int((maxs[list(group)].max(0) - mins[list(group)].min(0) + 1).sum())

    cands = []
    allseq = set(range(B))
    for combo in combinations(range(1, B), NCORES - 1):
        g0 = (0,) + combo
        g1 = tuple(sorted(allseq - set(g0)))
        cands.append((_npairs(g0) + _npairs(g1), (g0, g1)))
    cands.sort()

    def _try(slot_seqs):
        sched = []
        for q in range(SPC):
            seqs = list(slot_seqs[q])
            js_of = []
            for i in range(NST):
                blk = segment_ids[seqs, i * P:(i + 1) * P]
                lo, hi = int(blk.min()), int(blk.max())
                js_of.append(list(range(lo // P, hi // P + 1)))
            first, last = {}, {}
            for i in range(NST):
                for j in js_of[i]:
                    first.setdefault(j, i)
                    last[j] = i
            # the PSUM accumulator pools have 4 slots each; more
            # simultaneously-open t-tiles would deadlock the tile scheduler
            maxopen = max(sum(1 for j in first if first[j] <= i <= last[j])
                          for i in range(NST))
            if maxopen > 3:
                return None
            sched.append((tuple(tuple(js) for js in js_of),
                          tuple(sorted(first.items())),
                          tuple(sorted(last.items()))))
        return tuple(sched)

    for _, slot_seqs in cands:
        sched = _try(slot_seqs)
        if sched is not None:
            return sched, slot_seqs
    raise RuntimeError("no slot partition fits 3 open PSUM accumulators")


def _build(sched):
    from contextlib import ExitStack
    import concourse.bacc as bacc
    import concourse.tile as tile
    import concourse.mybir as mybir

    f32, f16 = mybir.dt.float32, mybir.dt.float16
    bf16, i32, f8 = mybir.dt.bfloat16, mybir.dt.int32, mybir.dt.float8e4
    u16 = mybir.dt.uint16
    AO = mybir.AluOpType
    DR = mybir.MatmulPerfMode.DoubleRow
    nc = bacc.Bacc("TRN2", target_bir_lowering=False, debug=False)
    # hi = fp8(x), lo = fp8(x - hi)
    xh_d = nc.dram_tensor("x_hi", [SPC, S, DW], f8, kind="ExternalInput").ap()
    xl_d = nc.dram_tensor("x_lo", [SPC, S, DW], f8, kind="ExternalInput").ap()
    # host-precomputed 1/max(count,1) per token: kills the on-device
    # count column + max/reciprocal chain that held PSUM banks open
    rec_d = nc.dram_tensor("rec", [SPC, P, NTT], f32,
                           kind="ExternalInput").ap()
    # host-precomputed one-hot ingredients, per s-tile column i:
    # sv[.., 0:NST]  srel = floor(sid/2) - 64 * first t-tile of i's window
    # sv[.., NST:]   vsel = 56 if sid even else 14336 (uint16 bit patterns
    #                of the packed fp8e4 pairs (1.0, 0) / (0, 1.0))
    sv_d = nc.dram_tensor("sv", [SPC, P, 2 * NST], f32,
                          kind="ExternalInput").ap()
    out = nc.dram_tensor("out", [SPC, T, D], bf16, kind="ExternalOutput").ap()

    with ExitStack() as ctx:
        tc = ctx.enter_context(tile.TileContext(nc))
        const = ctx.enter_context(tc.tile_pool(name="const", bufs=1))
        xp = ctx.enter_context(tc.tile_pool(name="xp", bufs=8))
        ohp = ctx.enter_context(tc.tile_pool(name="ohp", bufs=12))
        outp = ctx.enter_context(tc.tile_pool(name="outp", bufs=3))
        smp = ctx.enter_context(tc.tile_pool(name="smp", bufs=4))
        psb = ctx.enter_context(tc.tile_pool(name="psb", bufs=4, space="PSUM"))

        maxw = P * max(len(js) for q in range(SPC) for js in sched[q][0])
        ws = const.tile([P, DSPLIT], bf16)
        nc.gpsimd.memset(ws[:], 0.0)
        iota_i = const.tile([P, maxw // 2], i32)
        nc.gpsimd.iota(iota_i[:], pattern=[[1, maxw // 2]], base=0,
                       channel_multiplier=0)
        iota2_h = const.tile([P, maxw // 2], f16)
        nc.vector.tensor_copy(iota2_h[:], iota_i[:])

        # dummy accumulation chain: ~2us of PE activity while the first x
        # tiles are still in flight trips the HAM activity monitor, so the
        # real matmuls start at 2.4 GHz instead of the 1.2 GHz cold clock
        wps = psb.tile([P, DSPLIT], f32, tag="psA", name="warm")
        for k in range(5):
            nc.tensor.matmul(wps[:], lhsT=ws[:, 0:P], rhs=ws[:],
                             start=(k == 0), stop=(k == 4))

        # one-hot ingredients come precomputed from the host: one small DMA
        # per slot instead of a sid-transpose + 7-op DVE chain on the
        # critical path to the first matmul
        srels, vsels, recs = [], [], []
        for q in range(SPC):
            sv_t = smp.tile([P, 2 * NST], f32, tag="sv", name=f"sv_{q}")
            nc.sync.dma_start(out=sv_t[:], in_=sv_d[q])
            srels.append(sv_t[:, 0:NST])
            vsels.append(sv_t[:, NST:2 * NST])
            rec_t = smp.tile([P, NTT], f32, tag="rec", name=f"rec_{q}")
            nc.sync.dma_start(out=rec_t[:], in_=rec_d[q])
            recs.append(rec_t)

        ctxs = []
        for q in range(SPC):
            js_of, first_t, last_t = sched[q]
            ctxs.append({
                "js_of": js_of, "first": dict(first_t), "last": dict(last_t),
                "srel": srels[q], "vsel": vsels[q], "rec": recs[q],
                "xh_seq": xh_d[q].rearrange("(n p) d -> p n d", p=P),
                "xl_seq": xl_d[q].rearrange("(n p) d -> p n d", p=P),
                "out_seq": out[q].rearrange("(n p) d -> p n d", p=P),
                "open_ps": {}, "pend_out": {}})

        def emit_group(q, g):
            c = ctxs[q]
            js_of, first, last = c["js_of"], c["first"], c["last"]
            srel, vsel = c["srel"], c["vsel"]
            open_ps, pend_out = c["open_ps"], c["pend_out"]
            xt = xp.tile([P, 2, SUPER, DW], f8, tag="xt", name=f"xt_q{q}_g{g}")
            if g == 0:
                # split the first group's loads so the first matmuls only
                # wait on half the data -- shortens the pipeline head
                h = SUPER // 2
                nc.sync.dma_start(out=xt[:, 0, 0:h],
                                  in_=c["xh_seq"][:, 0:h, :])
                nc.sync.dma_start(out=xt[:, 1, 0:h],
                                  in_=c["xl_seq"][:, 0:h, :])
                nc.sync.dma_start(out=xt[:, 0, h:SUPER],
                                  in_=c["xh_seq"][:, h:SUPER, :])
                nc.sync.dma_start(out=xt[:, 1, h:SUPER],
                                  in_=c["xl_seq"][:, h:SUPER, :])
            else:
                nsl = slice(g * SUPER, (g + 1) * SUPER)
                nc.sync.dma_start(out=xt[:, 0], in_=c["xh_seq"][:, nsl, :])
                nc.sync.dma_start(out=xt[:, 1], in_=c["xl_seq"][:, nsl, :])
            # one-hots first: they only depend on the sid precompute, so
            # the DVE can produce them while the x DMA is still in flight
            ohws = []
            for l in range(SUPER):
                i = g * SUPER + l
                hw = P * len(js_of[i]) // 2
                ohw = ohp.tile([P, hw], u16, tag="oh", name=f"oh_q{q}_i{i}")
                nc.vector.tensor_scalar(
                    ohw[:], iota2_h[:, 0:hw], srel[:, i:i + 1],
                    vsel[:, i:i + 1], AO.is_equal, AO.mult)
                ohws.append(ohw)
            # one fp8 DoubleRow matmul pair per (s_tile, t_tile): Ko walks
            # the hi/lo planes of x; the one-hot broadcasts across Ko
            for l in range(SUPER):
                i = g * SUPER + l
                for k, j in enumerate(js_of[i]):
                    st = first[j] == i
                    sp_ = last[j] == i
                    if st:
                        open_ps[j] = (
                            psb.tile([P, DSPLIT], f32, tag="psA",
                                     name=f"accA_q{q}_j{j}"),
                            psb.tile([P, DW - DSPLIT], f32, tag="psB",
                                     name=f"accB_q{q}_j{j}"))
                    pa, pb = open_ps[j]
                    oh = (ohws[l][:, k * P // 2:(k + 1) * P // 2]
                          .bitcast(f8)
                          .rearrange("p (o w) -> p o w", o=1)
                          .broadcast_to((P, 2, P)))
                    nc.tensor.matmul(pa[:], lhsT=oh,
                                     rhs=xt[:, :, l, 0:DSPLIT],
                                     start=st, stop=sp_, perf_mode=DR)
                    nc.tensor.matmul(pb[:], lhsT=oh,
                                     rhs=xt[:, :, l, DSPLIT:DW],
                                     start=st, stop=sp_, perf_mode=DR)
                    if sp_:
                        rec = c["rec"][:, j:j + 1]
                        jp = j // OG
                        if jp not in pend_out:
                            ot = outp.tile([P, OG, D], bf16, tag="ot",
                                           name=f"ot_q{q}_{jp}")
                            need = 0
                            for m in range(OG):
                                if jp * OG + m in first:
                                    need += 1
                                else:
                                    nc.vector.memset(ot[:, m, :], 0.0)
                            pend_out[jp] = [ot, need]
                        ot, _ = pend_out[jp]
                        sl = j % OG
                        nc.scalar.activation(ot[:, sl, 0:DSPLIT], pa[:],
                                             mybir.ActivationFunctionType.Copy,
                                             scale=rec[:])
                        nc.vector.tensor_scalar_mul(
                            ot[:, sl, DSPLIT:D], pb[:, 0:D - DSPLIT], rec[:])
                        pend_out[jp][1] -= 1
                        if pend_out[jp][1] == 0:
                            nc.scalar.dma_start(
                                out=c["out_seq"][:, OG * jp:OG * (jp + 1), :],
                                in_=ot[:])
                            del pend_out[jp]
                        del open_ps[j]

        # interleave the two slots' groups: two independent dependency
        # chains keep every engine fed through the other chain's stalls
        for g in range(NST // SUPER):
            for q in range(SPC):
                emit_group(q, g)

        for q in range(SPC):
            c = ctxs[q]
            first, out_seq = c["first"], c["out_seq"]
            assert not c["pend_out"], "output group left pending"
            # output groups no s-tile can touch: store zeros
            for jp in range(NTT // OG):
                if all(jp * OG + m not in first for m in range(OG)):
                    zt = outp.tile([P, OG, D], bf16, tag="ot",
                                   name=f"zt_q{q}_{jp}")
                    nc.vector.memset(zt[:], 0.0)
                    nc.scalar.dma_start(
                        out=out_seq[:, OG * jp:OG * (jp + 1), :], in_=zt[:])
    nc.compile()
    return nc


def _get_nc(segment_ids: np.ndarray):
    sched, slot_seqs = _schedule(segment_ids)
    if sched not in _cache:
        _cache[sched] = _build(sched)
    return _cache[sched], slot_seqs, sched


def run(raw_output, segment_ids, trace=False):
    import ml_dtypes
    from concourse.bass_utils import run_bass_kernel_spmd

    f8 = ml_dtypes.float8_e4m3
    raw_output = np.asarray(raw_output, dtype=np.float32)
    segment_ids = np.ascontiguousarray(segment_ids, dtype=np.int32)
    nc, slot_seqs, sched = _get_nc(segment_ids)
    x_hi = raw_output.astype(f8)
    x_lo = (raw_output - x_hi.astype(np.float32)).astype(f8)
    wb = np.empty((SPC, NST), dtype=np.int64)
    for q in range(SPC):
        js_of = sched[q][0]
        for i in range(NST):
            wb[q, i] = js_of[i][0] * 64
    in_maps = []
    for c in range(NCORES):
        seqs = [slot_seqs[q][c] for q in range(SPC)]
        sv = np.empty((SPC, P, 2 * NST), dtype=np.float32)
        rec = np.empty((SPC, P, NTT), dtype=np.float32)
        for q in range(SPC):
            sidr = segment_ids[seqs[q]].reshape(NST, P)        # [i, p]
            sv[q, :, 0:NST] = (sidr // 2 - wb[q][:, None]).T
            sv[q, :, NST:] = np.where(sidr % 2 == 0, 56.0, 14336.0).T
            cnt = np.bincount(segment_ids[seqs[q]], minlength=T)
            rec[q] = (1.0 / np.maximum(cnt, 1)).reshape(NTT, P).T
        in_maps.append({
            "x_hi": np.ascontiguousarray(x_hi[seqs]),
            "x_lo": np.ascontiguousarray(x_lo[seqs]),
            "sv": sv, "rec": rec.astype(np.float32)})
    bkr = run_bass_kernel_spmd(nc, in_maps, list(range(NCORES)), trace=trace)
    full = np.empty((B, T, D), np.float32)
    for c in range(NCORES):
        for q in range(SPC):
            full[slot_seqs[q][c]] = bkr.results[c]["out"][q].astype(np.float32)
    return full, bkr


def kernel(raw_output, segment_ids):
    full, _ = run(raw_output, segment_ids,
                  trace=bool(int(os.environ.get("KERNEL_TRACE", "0"))))
    return full
